# revision 1
# baseline (speedup 1.0000x reference)
"""KAN layer kernel for Trainium2 (8 NeuronCores, data-parallel over batch).

Math (per feature d, hidden unit h):
    u[b,d] = sum_h W2[d,h] * relu(x[b,d]*W1[d,h] + b1[d,h]) + b2[d]
    out    = u @ Wc.T + bc

Strategy per core (B_local = 2048 batch rows, transposed layout [feature, batch]):
  - Hidden "hinge" terms are produced as [128, 2048] tiles where the 128
    partitions pack G=4 hidden units x 32 features (a feature band).
      * VectorE tiles:  m = max(x*W1, -b1)            (one tensor_scalar, 4x bf16)
        (relu(z) = max(W1*x, -b1) + b1; the +b1 constant is folded into the
        combiner bias on the host)
      * ScalarE tiles:  t = relu(x*W1 + b1)           (one activation inst)
  - TensorE contracts hidden units via matmuls whose stationary weights are
    4 stacked 32x32 diagonal blocks of W2, accumulating u in PSUM.
    Column-tiled (tile_position=(0,32j)) matmuls for the 4 feature bands run
    concurrently on the PE array.
  - Combiner: u (bf16) @ Wc.T blocks on TensorE, bias added by ScalarE.

All parameter preprocessing happens on the host in numpy; only x and out move
per-core. Output is computed transposed [O, B_local] and untransposed on host.
"""

import os
import numpy as np
import ml_dtypes

import concourse.bass as bass
import concourse.bacc as bacc
import concourse.tile as tile
import concourse.mybir as mybir
from concourse.bass_utils import run_bass_kernel_spmd

BF16 = ml_dtypes.bfloat16

B, D, H, O = 16384, 256, 64, 256
NCORES = 8
BL = B // NCORES          # 2048 batch rows per core
G = 4                     # hidden units packed per producer tile (row groups)
NQ = H // G               # 16 quads
NJ = 128 // 32            # 4 feature bands per 128-feature block
NDBLK = D // 128          # 2 feature blocks
F = BL                    # producer tile free size
MMF = 512                 # matmul moving chunk (one PSUM bank of fp32)
N_ACT = 13                # tiles per dblk produced on ScalarE (rest on VectorE)

_dt = mybir.dt


def _act_tile(q: int, j: int) -> bool:
    # Which producer tiles go to ScalarE (relu form); rest go VectorE (max form).
    return j == 3 and q < N_ACT


_NC_CACHE = None


def _build_nc():
    """Build + compile the Bass program once (same NEFF for all 8 cores)."""
    nc = bacc.Bacc("TRN2", target_bir_lowering=False, debug=False)

    xrep_d = nc.dram_tensor("xrep", [128, NDBLK * NJ * F], _dt.bfloat16,
                            kind="ExternalInput")
    sc1_d = nc.dram_tensor("sc1", [128, 128], _dt.float32, kind="ExternalInput")
    sc2_d = nc.dram_tensor("sc2", [128, 128], _dt.float32, kind="ExternalInput")
    wq_d = nc.dram_tensor("wq", [128, 128 * 32], _dt.bfloat16, kind="ExternalInput")
    wc_d = nc.dram_tensor("wc", [128, 4 * 128], _dt.bfloat16, kind="ExternalInput")
    bf_d = nc.dram_tensor("biasf", [128, 2], _dt.float32, kind="ExternalInput")
    out_d = nc.dram_tensor("outT", [O, BL], _dt.float32, kind="ExternalOutput")

    AF = mybir.ActivationFunctionType
    ALU = mybir.AluOpType

    with tile.TileContext(nc) as tc:
        with (
            tc.tile_pool(name="const", bufs=1) as cpool,
            tc.tile_pool(name="mpool", bufs=16) as mpool,
            tc.tile_pool(name="usb", bufs=1) as upool,
            tc.tile_pool(name="osb", bufs=1) as opool,
        ):
            xrep = cpool.tile([128, NDBLK * NJ * F], _dt.bfloat16, tag="xrep")
            sc1 = cpool.tile([128, 128], _dt.float32, tag="sc1")
            sc2 = cpool.tile([128, 128], _dt.float32, tag="sc2")
            wq = cpool.tile([128, 128 * 32], _dt.bfloat16, tag="wq")
            wc = cpool.tile([128, 4 * 128], _dt.bfloat16, tag="wc")
            bf = cpool.tile([128, 2], _dt.float32, tag="bf")

            nc.sync.dma_start(xrep[:], xrep_d[:])
            nc.sync.dma_start(sc1[:], sc1_d[:])
            nc.sync.dma_start(sc2[:], sc2_d[:])
            nc.sync.dma_start(wq[:], wq_d[:])
            nc.sync.dma_start(wc[:], wc_d[:])
            nc.sync.dma_start(bf[:], bf_d[:])

            u_sb = [upool.tile([128, F], _dt.bfloat16, tag=f"usb{i}", name=f"usb{i}")
                    for i in range(NDBLK)]

            # Zero weights for the PSUM-clearing dummy matmuls.
            zw = cpool.tile([128, 128], _dt.bfloat16, tag="zw")
            nc.vector.memset(zw[:], 0.0)

            with tc.tile_pool(name="upsum", bufs=1,
                              space=bass.MemorySpace.PSUM) as upsum:
                u_ps = [upsum.tile([128, F], _dt.float32, tag=f"ups{i}", name=f"ups{i}")
                        for i in range(NDBLK)]
                for dblk in range(NDBLK):
                    # One full-width start=True matmul per bank zeroes it (and
                    # sets has_written across all 128 partitions), so the
                    # partition-sliced accumulating matmuls below can all run
                    # with start=False in any interleaving.
                    for c in range(F // MMF):
                        nc.tensor.matmul(
                            u_ps[dblk][:, c * MMF:(c + 1) * MMF],
                            zw[:], xrep[:, 0:MMF],
                            start=True, stop=False, skip_group_check=True)
                    for q in range(NQ):
                        for j in range(NJ):
                            t = dblk * 64 + q * 4 + j
                            m = mpool.tile([128, F], _dt.bfloat16, tag="m", name=f"m{t}")
                            src = xrep[:, (dblk * NJ + j) * F:(dblk * NJ + j + 1) * F]
                            if _act_tile(q, j):
                                nc.scalar.activation(
                                    m[:], src, AF.Relu,
                                    bias=sc2[:, t:t + 1], scale=sc1[:, t:t + 1])
                            else:
                                nc.vector.tensor_scalar(
                                    m[:], src, sc1[:, t:t + 1], sc2[:, t:t + 1],
                                    ALU.mult, ALU.max)
                            for c in range(F // MMF):
                                r = nc.tensor.matmul(
                                    u_ps[dblk][32 * j:32 * j + 32,
                                               c * MMF:(c + 1) * MMF],
                                    wq[:, t * 32:(t + 1) * 32],
                                    m[:, c * MMF:(c + 1) * MMF],
                                    start=False, stop=(q == NQ - 1),
                                    tile_position=(0, 32 * j),
                                    skip_group_check=True)
                                if c > 0:
                                    # chunks 1-3 reuse the weights self-loaded
                                    # by chunk 0 (same readiness trigger, lower
                                    # priority => scheduled after it)
                                    r.ins.ldweights = False
                    nc.scalar.copy(u_sb[dblk][:], u_ps[dblk][:])

            out_sb = [opool.tile([128, F], _dt.float32, tag=f"o{i}", name=f"o{i}")
                      for i in range(2)]
            with tc.tile_pool(name="opsum", bufs=4,
                              space=bass.MemorySpace.PSUM) as opsum:
                for oblk in range(2):
                    opss = [opsum.tile([128, MMF], _dt.float32, tag="ops",
                                       name=f"ops{oblk}_{c}")
                            for c in range(F // MMF)]
                    for dblk in range(NDBLK):
                        for c in range(F // MMF):
                            r = nc.tensor.matmul(
                                opss[c][:],
                                wc[:, (dblk * 2 + oblk) * 128:
                                      (dblk * 2 + oblk + 1) * 128],
                                u_sb[dblk][:, c * MMF:(c + 1) * MMF],
                                start=(dblk == 0), stop=(dblk == NDBLK - 1))
                            if c > 0:
                                r.ins.ldweights = False
                    for c in range(F // MMF):
                        nc.scalar.activation(
                            out_sb[oblk][:, c * MMF:(c + 1) * MMF], opss[c][:],
                            AF.Identity, bias=bf[:, oblk:oblk + 1], scale=1.0)
                    nc.sync.dma_start(out_d[oblk * 128:(oblk + 1) * 128, :],
                                      out_sb[oblk][:])

    nc.compile()
    return nc


def _pack_params(W1, b1, W2, b2, Wc, bc):
    """Host-side packing of all parameter tensors (shared across cores)."""
    sc1 = np.zeros((128, 128), np.float32)
    sc2 = np.zeros((128, 128), np.float32)
    wq = np.zeros((128, 128 * 32), np.float32)
    K = np.zeros(D, np.float32)  # folded constants from the max-trick tiles

    for dblk in range(NDBLK):
        for q in range(NQ):
            for j in range(NJ):
                t = dblk * 64 + q * 4 + j
                d_vec = 128 * dblk + 32 * j + np.arange(32)
                is_act = _act_tile(q, j)
                for r in range(G):
                    h = G * q + r
                    rows = slice(32 * r, 32 * r + 32)
                    sc1[rows, t] = W1[d_vec, h]
                    sc2[rows, t] = b1[d_vec, h] if is_act else -b1[d_vec, h]
                    wq[rows, t * 32:(t + 1) * 32] = np.diag(W2[d_vec, h])
                    if not is_act:
                        K[d_vec] += W2[d_vec, h] * b1[d_vec, h]

    wc = np.zeros((128, 4 * 128), np.float32)
    for dblk in range(NDBLK):
        for oblk in range(2):
            blk = dblk * 2 + oblk
            wc[:, blk * 128:(blk + 1) * 128] = \
                Wc[oblk * 128:(oblk + 1) * 128, dblk * 128:(dblk + 1) * 128].T

    biasf = (bc + Wc @ (b2 + K)).astype(np.float32)
    bf = np.stack([biasf[:128], biasf[128:]], axis=1).copy()  # [128, 2]

    return {
        "sc1": sc1,
        "sc2": sc2,
        "wq": wq.astype(BF16),
        "wc": wc.astype(BF16),
        "biasf": bf,
    }


def _pack_x(x_core):
    """x_core [BL, D] fp32 -> replicated transposed bf16 [128, NDBLK*NJ*F]."""
    xT = np.ascontiguousarray(x_core.T).astype(BF16)  # [D, BL]
    xrep = np.empty((128, NDBLK * NJ * F), BF16)
    for dblk in range(NDBLK):
        for j in range(NJ):
            band = xT[128 * dblk + 32 * j:128 * dblk + 32 * j + 32, :]
            xrep[:, (dblk * NJ + j) * F:(dblk * NJ + j + 1) * F] = \
                np.tile(band, (G, 1))
    return xrep


LAST_RESULTS = None  # BassKernelResults of the most recent run (for profiling)


def kernel(x, W1, b1, W2, b2, Wc, bc):
    global _NC_CACHE, LAST_RESULTS
    x = np.asarray(x, np.float32)
    W1 = np.asarray(W1, np.float32)
    b1 = np.asarray(b1, np.float32)
    W2 = np.asarray(W2, np.float32)
    b2 = np.asarray(b2, np.float32)
    Wc = np.asarray(Wc, np.float32)
    bc = np.asarray(bc, np.float32)

    if _NC_CACHE is None:
        _NC_CACHE = _build_nc()
    nc = _NC_CACHE

    params = _pack_params(W1, b1, W2, b2, Wc, bc)
    in_maps = []
    for c in range(NCORES):
        m = dict(params)
        m["xrep"] = _pack_x(x[c * BL:(c + 1) * BL, :])
        in_maps.append(m)

    res = run_bass_kernel_spmd(nc, in_maps, core_ids=list(range(NCORES)))
    LAST_RESULTS = res

    out = np.empty((B, O), np.float32)
    for c in range(NCORES):
        out[c * BL:(c + 1) * BL, :] = res.results[c]["outT"].T
    return out


def _np_reference(x, W1, b1, W2, b2, Wc, bc):
    # numpy mirror of the oracle, used only for the __main__ sim self-check
    h = np.maximum(x[:, :, None] * W1[None] + b1[None], 0.0)
    u = np.einsum("bdh,dh->bd", h, W2) + b2[None, :]
    return u @ Wc.T + bc[None, :]


if __name__ == "__main__":
    # CoreSim self-check on a single core's worth of data (no hardware).
    from concourse.bass_interp import CoreSim

    rng = np.random.default_rng(0)
    x = rng.standard_normal((B, D)).astype(np.float32)
    W1 = rng.uniform(-1, 1, (D, H)).astype(np.float32)
    b1 = rng.uniform(-1, 1, (D, H)).astype(np.float32)
    W2 = rng.uniform(-0.125, 0.125, (D, H)).astype(np.float32)
    b2 = rng.uniform(-0.125, 0.125, (D,)).astype(np.float32)
    Wc = rng.uniform(-1 / 16, 1 / 16, (O, D)).astype(np.float32)
    bc = rng.uniform(-1 / 16, 1 / 16, (O,)).astype(np.float32)

    nc = _build_nc()
    params = _pack_params(W1, b1, W2, b2, Wc, bc)
    sim = CoreSim(nc)
    for k, v in params.items():
        sim.tensor(k)[:] = v
    sim.tensor("xrep")[:] = _pack_x(x[:BL, :])
    sim.simulate()
    got = np.asarray(sim.tensor("outT")).T

    want = _np_reference(x[:BL], W1, b1, W2, b2, Wc, bc)
    err = np.abs(got - want)
    rel = err.max() / (np.abs(want).max() + 1e-12)
    print(f"sim check: max abs err {err.max():.3e}  "
          f"rel-to-absmax {rel:.3e}  (|want| max {np.abs(want).max():.3f})")

